# revision 11
# baseline (speedup 1.0000x reference)
"""Trainium2 Bass kernel for nn_MCGRMRetrieval (sparse segment-gated retrieval).

Contract: kernel(**inputs) takes the FULL unsharded numpy inputs (keys as in
setup_inputs()) and returns the FULL outputs (h_retrieved, gate_current,
attn_weights), matching the jax reference.

Sharding: 8 NeuronCores, data parallel. Core c handles batch b=c//2 and 8 of
the 16 128-token blocks of that batch (interleaved assignment so the
block-causal retrieval work is balanced between the two cores of a batch).

Per-core pipeline (single SPMD Bass/Tile program, all matmuls bf16):
  A) uT[f,t] = sum_d W_u[f,d] x[t,d]             (PE, accumulated in PSUM)
  B) scores[t,n] vs m_stack, current-score via a ones-matmul, mask added as a
     data tensor (0/-1e30), 33-way softmax on DVE/ACT -> attn weights
  C) gates transposed (PE), then broadcast across 64 partitions via a K=2
     matmul with a 0/1 pattern as stationary operand -> e_bcast[k] (128,T)
  D) wphi = phiT_dup * e_bcast (DVE bf16 2x) ; retrieval matmuls with the
     paired segment states as stationary weights accumulate the gated sum
     over segment-pairs directly in PSUM; two heads run concurrently via
     column tiling. Pair k only covers the token suffix where it can be
     visible; masked gates are exactly 0 so over-coverage is exact.
"""

import sys

if "/opt/trn_rl_repo" not in sys.path:
    sys.path.insert(0, "/opt/trn_rl_repo")

import numpy as np

from concourse import bass, bacc, mybir, tile
from concourse.bass_utils import run_bass_kernel_spmd

# Problem constants (hardcoded per contract).
B, T, N, D = 4, 2048, 32, 1024
H, DK, DV = 16, 64, 64
SEG = T // N            # 64
BLK = 128               # token block (2 segments)
NBLK = T // BLK         # 16
NCORES = 8
NLOC = NBLK // 2        # 8 local blocks per core
TLOC = NLOC * BLK       # 1024 tokens per core
NPAIR_MAX = N // 2      # 16 segment pairs
SCALE = 1.0 / np.sqrt(D)

BLOCKS_EVEN = [0, 3, 4, 7, 8, 11, 12, 15]
BLOCKS_ODD = [1, 2, 5, 6, 9, 10, 13, 14]

F32 = mybir.dt.float32
BF16 = mybir.dt.bfloat16
NPBF = mybir.dt.np(BF16)

_CACHE = {}


def _schedule_from_mask(mask: np.ndarray):
    """Derive the SPMD pair schedule from the causal mask.

    Returns (npair, lstart) where pair k's retrieval matmuls cover local
    blocks L >= lstart[k]. Shared by all cores (program is identical)."""
    km = np.zeros(NBLK, np.int64)  # pairs needed per global block
    for m in range(NBLK):
        vis = np.nonzero(mask[m * BLK:(m + 1) * BLK].any(axis=0))[0]
        km[m] = 0 if len(vis) == 0 else (vis.max() // 2) + 1
    nv = np.maximum(km[BLOCKS_EVEN], km[BLOCKS_ODD])
    nv = np.maximum.accumulate(nv)  # nondecreasing over sorted local blocks
    npair = int(nv[-1])
    lstart = []
    for k in range(npair):
        ls = int(np.argmax(nv > k))
        lstart.append(0 if k == 0 else ls)
    return npair, tuple(lstart)


def _build_program(npair, lstart):
    nc = bacc.Bacc("TRN2", target_bir_lowering=False, debug=False)

    # ---- I/O declarations (dram layout == sbuf layout; host pre-packs) ----
    xT_d = nc.dram_tensor("xT", (128, 8, TLOC), BF16, kind="ExternalInput")
    wuT_d = nc.dram_tensor("wuT", (128, 8, D), BF16, kind="ExternalInput")
    mcT_d = nc.dram_tensor("mcT", (128, 8, TLOC), BF16, kind="ExternalInput")
    msT_d = nc.dram_tensor("msT", (128, 8, N), BF16, kind="ExternalInput")
    phiT_d = nc.dram_tensor("phiT", (128, H, TLOC), BF16, kind="ExternalInput")
    sp_d = nc.dram_tensor("sp", (128, H // 2, NPAIR_MAX, 128), BF16,
                          kind="ExternalInput")
    mb_d = nc.dram_tensor("maskb", (128, NLOC, N + 1), F32, kind="ExternalInput")
    pat_d = nc.dram_tensor("pat", (32, NPAIR_MAX, 128), BF16, kind="ExternalInput")
    id_d = nc.dram_tensor("ident", (128, 128), BF16, kind="ExternalInput")
    on_d = nc.dram_tensor("ones1", (128, 1), BF16, kind="ExternalInput")

    hT_d = nc.dram_tensor("hT", (H // 2, 128, TLOC), F32, kind="ExternalOutput")
    at_d = nc.dram_tensor("attn", (128, NLOC, N + 1), F32, kind="ExternalOutput")

    with tile.TileContext(nc) as tc:
        with (
            tc.tile_pool(name="res", bufs=1) as res,
            tc.tile_pool(name="shx", bufs=1) as shx,
            tc.tile_pool(name="wp", bufs=4) as wp,
            tc.tile_pool(name="hs", bufs=2) as hs,
            tc.tile_pool(name="sm", bufs=3) as sm,
            tc.tile_pool(name="psA", bufs=2, space=bass.MemorySpace.PSUM) as psA,
            tc.tile_pool(name="psS", bufs=2, space=bass.MemorySpace.PSUM) as psS,
            tc.tile_pool(name="psH", bufs=4, space=bass.MemorySpace.PSUM) as psH,
        ):
            # Resident inputs. xT shares its (oversized) slot with eb: xT is
            # dead after phase A, eb is first written in phase C.
            xT = shx.tile([128, 8, TLOC], BF16, tag="shx")
            wuT = res.tile([128, 8, D], BF16, tag="wuT")
            mcT = res.tile([128, 8, TLOC], BF16, tag="mcT")
            msT = res.tile([128, 8, N], BF16, tag="msT")
            phiT = res.tile([128, H, TLOC], BF16, tag="phiT")
            sp = res.tile([128, H // 2, NPAIR_MAX, 128], BF16, tag="sp")
            mb = res.tile([128, NLOC, N + 1], F32, tag="mb")
            pat = res.tile([32, NPAIR_MAX, 128], BF16, tag="pat")
            ident = res.tile([128, 128], BF16, tag="ident")
            ones1 = res.tile([128, 1], BF16, tag="ones1")

            uT = res.tile([128, 8, TLOC], BF16, tag="uT")
            pp = res.tile([128, 8, TLOC], BF16, tag="pp")
            eT = res.tile([32, TLOC], BF16, tag="eT")
            attn_sb = res.tile([128, NLOC, N + 1], F32, tag="attn_sb")

            nc.sync.dma_start(xT[:], xT_d[:])
            nc.sync.dma_start(wuT[:], wuT_d[:])
            nc.sync.dma_start(mcT[:], mcT_d[:])
            nc.sync.dma_start(msT[:], msT_d[:])
            nc.sync.dma_start(phiT[:], phiT_d[:])
            nc.sync.dma_start(sp[:], sp_d[:])
            nc.sync.dma_start(mb[:], mb_d[:])
            nc.sync.dma_start(pat[:], pat_d[:])
            nc.sync.dma_start(ident[:], id_d[:])
            nc.sync.dma_start(ones1[:], on_d[:])

            # ---- Phase A: uT[f,t] = sum_d W_u[f,d] x[t,d] ----
            for tc2 in range(2):
                tsl = slice(tc2 * 512, tc2 * 512 + 512)
                for f in range(8):
                    ps = psA.tile([128, 512], F32, tag="mm512")
                    for d in range(8):
                        nc.tensor.matmul(
                            ps[:], wuT[:, d, f * 128:(f + 1) * 128],
                            xT[:, d, tsl], start=(d == 0), stop=(d == 7))
                    nc.scalar.copy(uT[:, f, tsl], ps[:])

            # ---- Phase B: scores, softmax, attn out ----
            for f in range(8):
                nc.vector.tensor_mul(pp[:, f, :], uT[:, f, :], mcT[:, f, :])
            for tb in range(NLOC):
                tbs = slice(tb * BLK, (tb + 1) * BLK)
                psc = psS.tile([128, 1], F32, tag="sm1")
                for f in range(8):
                    nc.tensor.matmul(psc[:], pp[:, f, tbs], ones1[:],
                                     start=(f == 0), stop=(f == 7))
                pss = psS.tile([128, N], F32, tag="sm1")
                for f in range(8):
                    nc.tensor.matmul(pss[:], uT[:, f, tbs], msT[:, f, :],
                                     start=(f == 0), stop=(f == 7))
                sc33 = sm.tile([128, N + 1], F32, tag="sc33")
                nc.vector.tensor_add(sc33[:, 0:N], pss[:], mb[:, tb, 0:N])
                nc.vector.tensor_copy(sc33[:, N:N + 1], psc[:])
                negmx = sm.tile([128, 1], F32, tag="negmx")
                nc.vector.reduce_max(negmx[:], sc33[:], mybir.AxisListType.X,
                                     negate=True)
                e33 = sm.tile([128, N + 1], F32, tag="e33")
                nc.scalar.activation(e33[:], sc33[:],
                                     mybir.ActivationFunctionType.Exp,
                                     bias=negmx[:], scale=1.0)
                zs = sm.tile([128, 1], F32, tag="zs")
                nc.vector.reduce_sum(zs[:], e33[:], mybir.AxisListType.X)
                invz = sm.tile([128, 1], F32, tag="invz")
                nc.vector.reciprocal(invz[:], zs[:])
                nc.vector.tensor_scalar_mul(attn_sb[:, tb, :], e33[:], invz[:])
                attb = sm.tile([128, N], BF16, tag="attb")
                nc.vector.tensor_scalar_mul(attb[:], e33[:, 0:N], invz[:])
                psT = psS.tile([32, 128], BF16, tag="sm1")
                nc.tensor.transpose(psT[:], attb[:], ident[:])
                nc.scalar.copy(eT[:, tbs], psT[:])
            nc.sync.dma_start(at_d[:], attn_sb[:])

            # ---- Phase C: broadcast gates to 128 partitions per pair ----
            eb = shx.tile([128, 16, TLOC], BF16, tag="shx")
            for k in range(npair):
                for tc2 in range(2):
                    tsl = slice(tc2 * 512, tc2 * 512 + 512)
                    ps = psA.tile([128, 512], F32, tag="mm512")
                    nc.tensor.matmul(ps[:], pat[:, k, :], eT[:, tsl])
                    nc.scalar.copy(eb[:, k, tsl], ps[:])

            # ---- Phase D: gated retrieval, PSUM-accumulated over pairs ----
            # last pair touching each 512-half (for stop flags)
            klast = [max(k for k in range(npair) if lstart[k] * BLK < 512 * (h2 + 1))
                     for h2 in range(2)]
            for hp in range(H // 2):
                ph = [psH.tile([128, 512], F32, tag="hT", name=f"ph{h2}")
                      for h2 in range(2)]
                for k in range(npair):
                    t0 = lstart[k] * BLK
                    w1 = wp.tile([128, TLOC - t0], BF16, tag="wphi")
                    w2 = wp.tile([128, TLOC - t0], BF16, tag="wphi")
                    nc.vector.tensor_mul(w1[:], phiT[:, 2 * hp, t0:TLOC],
                                         eb[:, k, t0:TLOC])
                    nc.vector.tensor_mul(w2[:], phiT[:, 2 * hp + 1, t0:TLOC],
                                         eb[:, k, t0:TLOC])
                    for h2 in range(2):
                        lo = max(t0, 512 * h2)
                        hi = 512 * (h2 + 1)
                        if lo >= hi:
                            continue
                        c0, cn, w0 = lo - 512 * h2, hi - lo, lo - t0
                        nc.tensor.matmul(
                            ph[h2][0:64, c0:c0 + cn],
                            sp[:, hp, k, 0:64], w1[:, w0:w0 + cn],
                            start=(k == 0), stop=(k == klast[h2]),
                            tile_position=(0, 0), skip_group_check=True)
                        nc.tensor.matmul(
                            ph[h2][64:128, c0:c0 + cn],
                            sp[:, hp, k, 64:128], w2[:, w0:w0 + cn],
                            start=(k == 0), stop=(k == klast[h2]),
                            tile_position=(0, 64), skip_group_check=True)
                hsb = hs.tile([128, TLOC], F32, tag="hsb")
                nc.scalar.copy(hsb[:, 0:512], ph[0][:])
                nc.scalar.copy(hsb[:, 512:1024], ph[1][:])
                nc.sync.dma_start(hT_d[hp], hsb[:])

    nc.compile()
    return nc


def _pack_inputs(x, phi_q, state_stack, m_stack, m_current, causal_mask, W_u):
    """Build the 8 per-core input dicts (host-side shard + layout)."""
    wuT = np.ascontiguousarray(W_u.T).reshape(8, 128, D)  # [dchunk, p, f]
    wuT = np.ascontiguousarray(wuT.transpose(1, 0, 2)).astype(NPBF)  # (128,8,D)
    # pat[n, k, m] selects gate row n=2k for output rows m<64, n=2k+1 for m>=64
    pat = np.zeros((32, NPAIR_MAX, 128), NPBF)
    for k in range(NPAIR_MAX):
        pat[2 * k, k, :64] = 1.0
        pat[2 * k + 1, k, 64:] = 1.0
    ident = np.eye(128, dtype=NPBF)
    ones1 = np.ones((128, 1), NPBF)

    in_maps = []
    for c in range(NCORES):
        b = c // 2
        blocks = BLOCKS_EVEN if c % 2 == 0 else BLOCKS_ODD
        tsel = np.concatenate([np.arange(m * BLK, (m + 1) * BLK) for m in blocks])

        def t_major(a2d):  # (TLOC, D) -> (128 part=dchunk-row, 8, TLOC)
            aT = np.ascontiguousarray(a2d.T)          # (D, TLOC)
            aT = aT.reshape(8, 128, TLOC)
            return np.ascontiguousarray(aT.transpose(1, 0, 2))

        xT = t_major(x[b][tsel]).astype(NPBF)
        mcT = t_major(m_current[b][tsel] * SCALE).astype(NPBF)
        msT = np.ascontiguousarray((m_stack[b].T * SCALE).reshape(8, 128, N)
                                   .transpose(1, 0, 2)).astype(NPBF)
        # phiT duplicated across the two 64-partition halves: (128, H, TLOC)
        ph = phi_q[b][tsel]                            # (TLOC, H, DK)
        phT = np.ascontiguousarray(ph.transpose(1, 2, 0))  # (H, DK, TLOC)
        phiT = np.concatenate([phT, phT], axis=1)      # (H, 128, TLOC)
        phiT = np.ascontiguousarray(phiT.transpose(1, 0, 2)).astype(NPBF)
        # sp[kk, hp, j, vv]
        S = state_stack[b]                             # (N, H, DK, DV)
        spv = np.zeros((128, H // 2, NPAIR_MAX, 128), np.float32)
        for j in range(NPAIR_MAX):
            for half in range(2):
                Sv = S[2 * j + half]                   # (H, DK, DV)
                kk = slice(64 * half, 64 * half + 64)
                for hh in range(2):
                    spv[kk, :, j, 64 * hh:64 * hh + 64] = \
                        Sv[hh::2].transpose(1, 0, 2)   # (DK, H/2, DV)
        sp = spv.astype(NPBF)
        # mask bias (0 / -1e30): (128 p=t-in-block, NLOC, 33)
        mbv = np.where(causal_mask[tsel], 0.0, -1e30).astype(np.float32)
        mbv = np.concatenate([mbv, np.zeros((TLOC, 1), np.float32)], axis=1)
        mb = np.ascontiguousarray(mbv.reshape(NLOC, BLK, N + 1).transpose(1, 0, 2))

        in_maps.append({
            "xT": xT, "wuT": wuT.copy(), "mcT": mcT, "msT": msT,
            "phiT": phiT, "sp": sp, "maskb": mb, "pat": pat.copy(),
            "ident": ident.copy(), "ones1": ones1.copy(),
        })
    return in_maps


def _unpack_outputs(results):
    h = np.zeros((B, T, H, DV), np.float32)
    attn = np.zeros((B, T, N + 1), np.float32)
    for c in range(NCORES):
        b = c // 2
        blocks = BLOCKS_EVEN if c % 2 == 0 else BLOCKS_ODD
        tsel = np.concatenate([np.arange(m * BLK, (m + 1) * BLK) for m in blocks])
        hT = results[c]["hT"]                     # (H/2, 128, TLOC)
        at = results[c]["attn"]                   # (128, NLOC, 33)
        # hT[hp, vv, t]: head = 2*hp + (vv >= 64), v = vv % 64
        hTr = hT.reshape(H // 2, 2, DV, TLOC)     # (hp, hh, v, t)
        h[b, tsel] = hTr.transpose(3, 0, 1, 2).reshape(TLOC, H, DV)
        attn[b, tsel] = at.transpose(1, 0, 2).reshape(TLOC, N + 1)
    hr = h.reshape(B, T, H * DV)
    gc = attn[:, :, N:N + 1].copy()
    return hr, gc, attn


def kernel(**inputs):
    mask = np.asarray(inputs["causal_mask"])
    sched = _schedule_from_mask(mask)
    if sched not in _CACHE:
        _CACHE[sched] = _build_program(*sched)
    nc = _CACHE[sched]
    in_maps = _pack_inputs(
        np.asarray(inputs["x"], np.float32),
        np.asarray(inputs["phi_q"], np.float32),
        np.asarray(inputs["state_stack"], np.float32),
        np.asarray(inputs["m_stack"], np.float32),
        np.asarray(inputs["m_current"], np.float32),
        mask,
        np.asarray(inputs["W_u"], np.float32),
    )
    res = run_bass_kernel_spmd(nc, in_maps, core_ids=list(range(NCORES)))
    return _unpack_outputs(res.results)


# revision 12
# speedup vs baseline: 1.0690x; 1.0690x over previous
"""Trainium2 Bass kernel for nn_MCGRMRetrieval (sparse segment-gated retrieval).

Contract: kernel(**inputs) takes the FULL unsharded numpy inputs (keys as in
setup_inputs()) and returns the FULL outputs (h_retrieved, gate_current,
attn_weights), matching the jax reference.

Sharding: 8 NeuronCores, data parallel. Core c handles batch b=c//2 and 8 of
the 16 128-token blocks of that batch (interleaved assignment so the
block-causal retrieval work is balanced between the two cores of a batch).

Per-core pipeline (single SPMD Bass/Tile program, all matmuls bf16):
  A) uT[f,t] = sum_d W_u[f,d] x[t,d] (PE); scores^T vs m_stack and the
     current-token score (ones-matmul) accumulate in a side PSUM tile,
     interleaved per f-chunk so the PE never idles.
  B) per token-block: PE-transpose scores^T -> (t,33), add the mask bias
     (a data tensor of 0/-1e30 so the program is mask-agnostic), 33-way
     softmax (DVE/ACT), transpose gates back -> eT (32,T).
  C) broadcast gate rows across 64 partitions via K=32 matmuls with 0/1
     selection patterns as stationary operands -> eb[k] (128,T) per pair.
  D) wphi = phiT_dup * eb (DVE bf16 2x, one op per (head-pair, pair, half)
     via a 0-stride broadcast AP); retrieval matmuls with the paired
     segment states as stationary weights accumulate the gated sum over
     pairs directly in PSUM; the two heads of a pair run concurrently via
     column tiling. Pair k only covers the token suffix where it can be
     visible; masked gates are exactly 0 so over-coverage is exact.
"""

import sys

if "/opt/trn_rl_repo" not in sys.path:
    sys.path.insert(0, "/opt/trn_rl_repo")

import numpy as np

from concourse import bass, bacc, mybir, tile
from concourse.bass_utils import run_bass_kernel_spmd

# Problem constants (hardcoded per contract).
B, T, N, D = 4, 2048, 32, 1024
H, DK, DV = 16, 64, 64
SEG = T // N            # 64
BLK = 128               # token block (2 segments)
NBLK = T // BLK         # 16
NCORES = 8
NLOC = NBLK // 2        # 8 local blocks per core
TLOC = NLOC * BLK       # 1024 tokens per core
NPAIR_MAX = N // 2      # 16 segment pairs
SCALE = 1.0 / np.sqrt(D)

BLOCKS_EVEN = [0, 3, 4, 7, 8, 11, 12, 15]
BLOCKS_ODD = [1, 2, 5, 6, 9, 10, 13, 14]

F32 = mybir.dt.float32
BF16 = mybir.dt.bfloat16
NPBF = mybir.dt.np(BF16)

_CACHE = {}


def _schedule_from_mask(mask: np.ndarray):
    """Derive the SPMD pair schedule from the causal mask.

    Returns (npair, lstart) where pair k's retrieval matmuls cover local
    blocks L >= lstart[k]. Shared by all cores (program is identical)."""
    km = np.zeros(NBLK, np.int64)  # pairs needed per global block
    for m in range(NBLK):
        vis = np.nonzero(mask[m * BLK:(m + 1) * BLK].any(axis=0))[0]
        km[m] = 0 if len(vis) == 0 else (vis.max() // 2) + 1
    nv = np.maximum(km[BLOCKS_EVEN], km[BLOCKS_ODD])
    nv = np.maximum.accumulate(nv)  # nondecreasing over sorted local blocks
    npair = int(nv[-1])
    lstart = []
    for k in range(npair):
        ls = int(np.argmax(nv > k))
        lstart.append(0 if k == 0 else ls)
    return npair, tuple(lstart)


def _build_program(npair, lstart):
    nc = bacc.Bacc("TRN2", target_bir_lowering=False, debug=False)

    # ---- I/O declarations (dram layout == sbuf layout; host pre-packs) ----
    xT_d = nc.dram_tensor("xT", (128, 8, TLOC), BF16, kind="ExternalInput")
    wuT_d = nc.dram_tensor("wuT", (128, 8, D), BF16, kind="ExternalInput")
    mcT_d = nc.dram_tensor("mcT", (128, 8, TLOC), BF16, kind="ExternalInput")
    msT_d = nc.dram_tensor("msT", (128, 8, N), BF16, kind="ExternalInput")
    phiT_d = nc.dram_tensor("phiT", (128, H, TLOC), BF16, kind="ExternalInput")
    sp_d = nc.dram_tensor("sp", (128, H // 2, NPAIR_MAX, 128), BF16,
                          kind="ExternalInput")
    mb_d = nc.dram_tensor("maskb", (128, NLOC, N + 1), F32, kind="ExternalInput")
    pat_d = nc.dram_tensor("pat", (32, NPAIR_MAX, 128), BF16, kind="ExternalInput")
    i33_d = nc.dram_tensor("ident33", (33, 33), F32, kind="ExternalInput")
    i128_d = nc.dram_tensor("ident", (128, 128), BF16, kind="ExternalInput")
    on_d = nc.dram_tensor("ones1", (128, 1), BF16, kind="ExternalInput")

    hT_d = nc.dram_tensor("hT", (H // 2, 128, TLOC), F32, kind="ExternalOutput")
    at_d = nc.dram_tensor("attn", (128, NLOC, N + 1), F32, kind="ExternalOutput")

    with tile.TileContext(nc) as tc:
        with (
            tc.tile_pool(name="res", bufs=1) as res,
            tc.tile_pool(name="shx", bufs=1) as shx,
            tc.tile_pool(name="wp", bufs=4) as wp,
            tc.tile_pool(name="hs", bufs=2) as hs,
            tc.tile_pool(name="sm", bufs=4) as sm,
            tc.tile_pool(name="psG", bufs=3, space=bass.MemorySpace.PSUM) as psG,
            tc.tile_pool(name="psSc", bufs=2, space=bass.MemorySpace.PSUM) as psSc,
            tc.tile_pool(name="psH", bufs=3, space=bass.MemorySpace.PSUM) as psH,
        ):
            # Resident inputs. xT shares its (oversized) slot with eb: xT is
            # dead after phase A, eb is first written in phase C.
            xT = shx.tile([128, 8, TLOC], BF16, tag="shx")
            wuT = res.tile([128, 8, D], BF16, tag="wuT")
            mcT = res.tile([128, 8, TLOC], BF16, tag="mcT")
            msT = res.tile([128, 8, N], BF16, tag="msT")
            phiT = res.tile([128, H, TLOC], BF16, tag="phiT")
            sp = res.tile([128, H // 2, NPAIR_MAX, 128], BF16, tag="sp")
            mb = res.tile([128, NLOC, N + 1], F32, tag="mb")
            pat = res.tile([32, NPAIR_MAX, 128], BF16, tag="pat")
            id33 = res.tile([33, 33], F32, tag="id33")
            id128 = res.tile([128, 128], BF16, tag="id128")
            ones1 = res.tile([128, 1], BF16, tag="ones1")

            uT = res.tile([128, 8, TLOC], BF16, tag="uT")
            pp = res.tile([128, 8, TLOC], BF16, tag="pp")
            scsT = res.tile([33, TLOC], F32, tag="scsT")
            eT = res.tile([32, TLOC], BF16, tag="eT")
            attn_sb = res.tile([128, NLOC, N + 1], F32, tag="attn_sb")

            nc.sync.dma_start(xT[:], xT_d[:])
            nc.sync.dma_start(wuT[:], wuT_d[:])
            nc.sync.dma_start(mcT[:], mcT_d[:])
            nc.sync.dma_start(msT[:], msT_d[:])
            nc.sync.dma_start(phiT[:], phiT_d[:])
            nc.sync.dma_start(sp[:], sp_d[:])
            nc.sync.dma_start(mb[:], mb_d[:])
            nc.sync.dma_start(pat[:], pat_d[:])
            nc.sync.dma_start(id33[:], i33_d[:])
            nc.sync.dma_start(id128[:], i128_d[:])
            nc.sync.dma_start(ones1[:], on_d[:])

            # ---- Phase A: u-projection + scores^T, interleaved per f ----
            pstA = [psSc.tile([33, 512], F32, tag="sct", name=f"pst{t2}")
                    for t2 in range(2)]
            for f in range(8):
                for tc2 in range(2):
                    tsl = slice(tc2 * 512, tc2 * 512 + 512)
                    ps = psG.tile([128, 512], F32, tag="g512")
                    for d in range(8):
                        nc.tensor.matmul(
                            ps[:], wuT[:, d, f * 128:(f + 1) * 128],
                            xT[:, d, tsl], start=(d == 0), stop=(d == 7))
                    nc.scalar.copy(uT[:, f, tsl], ps[:])
                nc.vector.tensor_mul(pp[:, f, :], uT[:, f, :], mcT[:, f, :])
                for tc2 in range(2):
                    tsl = slice(tc2 * 512, tc2 * 512 + 512)
                    nc.tensor.matmul(
                        pstA[tc2][0:32, :], msT[:, f, :], uT[:, f, tsl],
                        start=(f == 0), stop=(f == 7),
                        tile_position=(0, 0), skip_group_check=True)
                    nc.tensor.matmul(
                        pstA[tc2][32:33, :], ones1[:], pp[:, f, tsl],
                        start=(f == 0), stop=(f == 7),
                        tile_position=(0, 32), skip_group_check=True)
            for tc2 in range(2):
                nc.scalar.copy(scsT[:, tc2 * 512:tc2 * 512 + 512], pstA[tc2][:])

            # ---- Phase B: per-block transpose + softmax + gate transpose ----
            for tb in range(NLOC):
                tbs = slice(tb * BLK, (tb + 1) * BLK)
                ps33 = psG.tile([128, N + 1], F32, tag="g512", name="ps33")
                nc.tensor.transpose(ps33[:], scsT[:, tbs], id33[:])
                sc33 = sm.tile([128, N + 1], F32, tag="sc33")
                nc.vector.tensor_add(sc33[:], ps33[:], mb[:, tb, :])
                negmx = sm.tile([128, 1], F32, tag="negmx")
                nc.vector.reduce_max(negmx[:], sc33[:], mybir.AxisListType.X,
                                     negate=True)
                e33 = sm.tile([128, N + 1], F32, tag="e33")
                nc.scalar.activation(e33[:], sc33[:],
                                     mybir.ActivationFunctionType.Exp,
                                     bias=negmx[:], scale=1.0)
                zs = sm.tile([128, 1], F32, tag="zs")
                nc.vector.reduce_sum(zs[:], e33[:], mybir.AxisListType.X)
                invz = sm.tile([128, 1], F32, tag="invz")
                nc.vector.reciprocal(invz[:], zs[:])
                nc.vector.tensor_scalar_mul(attn_sb[:, tb, :], e33[:], invz[:])
                attb = sm.tile([128, N], BF16, tag="attb")
                nc.vector.tensor_scalar_mul(attb[:], e33[:, 0:N], invz[:])
                psTe = psG.tile([32, 128], BF16, tag="g512", name="psTe")
                nc.tensor.transpose(psTe[:], attb[:], id128[:])
                nc.scalar.copy(eT[:, tbs], psTe[:])
            nc.sync.dma_start(at_d[:], attn_sb[:])

            # ---- Phase C: broadcast gates to 128 partitions per pair ----
            eb = shx.tile([128, 16, TLOC], BF16, tag="shx")
            for tc2 in range(2):
                tsl = slice(tc2 * 512, tc2 * 512 + 512)
                for k in range(npair):
                    ps = psG.tile([128, 512], F32, tag="g512", name="psbc")
                    nc.tensor.matmul(ps[:], pat[:, k, :], eT[:, tsl])
                    nc.scalar.copy(eb[:, k, tsl], ps[:])

            # ---- Phase D: gated retrieval, PSUM-accumulated over pairs ----
            # last pair touching each 512-half (for stop flags)
            klast = [max(k for k in range(npair) if lstart[k] * BLK < 512 * (h2 + 1))
                     for h2 in range(2)]
            for hp in range(H // 2):
                ph = [psH.tile([128, 512], F32, tag="hT", name=f"ph{h2}")
                      for h2 in range(2)]
                for k in range(npair):
                    t0 = lstart[k] * BLK
                    for h2 in range(2):
                        lo = max(t0, 512 * h2)
                        hi = 512 * (h2 + 1)
                        if lo >= hi:
                            continue
                        c0, cn = lo - 512 * h2, hi - lo
                        w12 = wp.tile([128, 2, cn], BF16, tag="wphi",
                                      name="w12")
                        ebb = eb[:, k, lo:hi] \
                            .rearrange("p (o t) -> p o t", o=1) \
                            .to_broadcast((128, 2, cn))
                        nc.vector.tensor_mul(
                            w12[:], phiT[:, 2 * hp:2 * hp + 2, lo:hi], ebb)
                        nc.tensor.matmul(
                            ph[h2][0:64, c0:c0 + cn],
                            sp[:, hp, k, 0:64], w12[:, 0, :],
                            start=(k == 0), stop=(k == klast[h2]),
                            tile_position=(0, 0), skip_group_check=True)
                        nc.tensor.matmul(
                            ph[h2][64:128, c0:c0 + cn],
                            sp[:, hp, k, 64:128], w12[:, 1, :],
                            start=(k == 0), stop=(k == klast[h2]),
                            tile_position=(0, 64), skip_group_check=True)
                hsb = hs.tile([128, TLOC], F32, tag="hsb")
                nc.scalar.copy(hsb[:, 0:512], ph[0][:])
                nc.scalar.copy(hsb[:, 512:1024], ph[1][:])
                nc.sync.dma_start(hT_d[hp], hsb[:])

    nc.compile()
    return nc


def _pack_inputs(x, phi_q, state_stack, m_stack, m_current, causal_mask, W_u):
    """Build the 8 per-core input dicts (host-side shard + layout)."""
    wuT = np.ascontiguousarray(W_u.T).reshape(8, 128, D)  # [dchunk, p, f]
    wuT = np.ascontiguousarray(wuT.transpose(1, 0, 2)).astype(NPBF)  # (128,8,D)
    # pat[n, k, m] selects gate row n=2k for output rows m<64, n=2k+1 for m>=64
    pat = np.zeros((32, NPAIR_MAX, 128), NPBF)
    for k in range(NPAIR_MAX):
        pat[2 * k, k, :64] = 1.0
        pat[2 * k + 1, k, 64:] = 1.0
    ident33 = np.eye(33, dtype=np.float32)
    ident = np.eye(128, dtype=NPBF)
    ones1 = np.ones((128, 1), NPBF)

    in_maps = []
    for c in range(NCORES):
        b = c // 2
        blocks = BLOCKS_EVEN if c % 2 == 0 else BLOCKS_ODD
        tsel = np.concatenate([np.arange(m * BLK, (m + 1) * BLK) for m in blocks])

        def t_major(a2d):  # (TLOC, D) -> (128 part=dchunk-row, 8, TLOC)
            aT = np.ascontiguousarray(a2d.T)          # (D, TLOC)
            aT = aT.reshape(8, 128, TLOC)
            return np.ascontiguousarray(aT.transpose(1, 0, 2))

        xT = t_major(x[b][tsel]).astype(NPBF)
        mcT = t_major(m_current[b][tsel] * SCALE).astype(NPBF)
        msT = np.ascontiguousarray((m_stack[b].T * SCALE).reshape(8, 128, N)
                                   .transpose(1, 0, 2)).astype(NPBF)
        # phiT duplicated across the two 64-partition halves: (128, H, TLOC)
        ph = phi_q[b][tsel]                            # (TLOC, H, DK)
        phT = np.ascontiguousarray(ph.transpose(1, 2, 0))  # (H, DK, TLOC)
        phiT = np.concatenate([phT, phT], axis=1)      # (H, 128, TLOC)
        phiT = np.ascontiguousarray(phiT.transpose(1, 0, 2)).astype(NPBF)
        # sp[kk, hp, j, vv]
        S = state_stack[b]                             # (N, H, DK, DV)
        spv = np.zeros((128, H // 2, NPAIR_MAX, 128), np.float32)
        for j in range(NPAIR_MAX):
            for half in range(2):
                Sv = S[2 * j + half]                   # (H, DK, DV)
                kk = slice(64 * half, 64 * half + 64)
                for hh in range(2):
                    spv[kk, :, j, 64 * hh:64 * hh + 64] = \
                        Sv[hh::2].transpose(1, 0, 2)   # (DK, H/2, DV)
        sp = spv.astype(NPBF)
        # mask bias (0 / -1e30): (128 p=t-in-block, NLOC, 33)
        mbv = np.where(causal_mask[tsel], 0.0, -1e30).astype(np.float32)
        mbv = np.concatenate([mbv, np.zeros((TLOC, 1), np.float32)], axis=1)
        mb = np.ascontiguousarray(mbv.reshape(NLOC, BLK, N + 1).transpose(1, 0, 2))

        in_maps.append({
            "xT": xT, "wuT": wuT.copy(), "mcT": mcT, "msT": msT,
            "phiT": phiT, "sp": sp, "maskb": mb, "pat": pat.copy(),
            "ident33": ident33.copy(), "ident": ident.copy(),
            "ones1": ones1.copy(),
        })
    return in_maps


def _unpack_outputs(results):
    h = np.zeros((B, T, H, DV), np.float32)
    attn = np.zeros((B, T, N + 1), np.float32)
    for c in range(NCORES):
        b = c // 2
        blocks = BLOCKS_EVEN if c % 2 == 0 else BLOCKS_ODD
        tsel = np.concatenate([np.arange(m * BLK, (m + 1) * BLK) for m in blocks])
        hT = results[c]["hT"]                     # (H/2, 128, TLOC)
        at = results[c]["attn"]                   # (128, NLOC, 33)
        # hT[hp, vv, t]: head = 2*hp + (vv >= 64), v = vv % 64
        hTr = hT.reshape(H // 2, 2, DV, TLOC)     # (hp, hh, v, t)
        h[b, tsel] = hTr.transpose(3, 0, 1, 2).reshape(TLOC, H, DV)
        attn[b, tsel] = at.transpose(1, 0, 2).reshape(TLOC, N + 1)
    hr = h.reshape(B, T, H * DV)
    gc = attn[:, :, N:N + 1].copy()
    return hr, gc, attn


def kernel(**inputs):
    mask = np.asarray(inputs["causal_mask"])
    sched = _schedule_from_mask(mask)
    if sched not in _CACHE:
        _CACHE[sched] = _build_program(*sched)
    nc = _CACHE[sched]
    in_maps = _pack_inputs(
        np.asarray(inputs["x"], np.float32),
        np.asarray(inputs["phi_q"], np.float32),
        np.asarray(inputs["state_stack"], np.float32),
        np.asarray(inputs["m_stack"], np.float32),
        np.asarray(inputs["m_current"], np.float32),
        mask,
        np.asarray(inputs["W_u"], np.float32),
    )
    res = run_bass_kernel_spmd(nc, in_maps, core_ids=list(range(NCORES)))
    return _unpack_outputs(res.results)
